# revision 42
# baseline (speedup 1.0000x reference)
"""BotRGCN forward on 8 Trainium2 NeuronCores.

Strategy (matches the sharding hint):
  - Nodes sharded across 8 cores (12500 rows each, padded to 12800 = 25*512).
  - Edges partitioned by destination-node owner; per core, edges are bucketed
    by (relation, src-bank, 128-node window of the destination) and packed
    into 128-edge chunks on the host.  Banks are <=32K-row ranges of the
    all-gathered x table so dma_gather's int16 indices can address them.
  - Segment-mean on the TensorEngine: for each 128-edge chunk, gather x[src]
    rows (dma_gather) -> one-hot(seg_local) built on the VectorEngine ->
    matmul accumulates sumT[d, node] into PSUM.  The 1/deg scaling is folded
    into the PSUM->SBUF copy via a host-precomputed recip tile.
  - Small weights replicated; x all-gathered across cores between layers.
  - Head (lrelu @ W_o1, @ W_o2) on-device; softmax on host.

The Bass program is identical across cores (SPMD); all per-core variation is
carried by input tensors.  The chunk schedule (chunks per window) is the max
over cores, so the program structure is consistent across cores.
"""

import sys

sys.path.insert(0, "/opt/trn_rl_repo")

import numpy as np
import ml_dtypes

import concourse.bass as bass
import concourse.mybir as mybir
import concourse.tile as tile
from concourse import bass_utils, library_config, library_overlay

# ---- fixed problem geometry -------------------------------------------------
NCORES = 8
D = 128          # hidden width
R = 2            # relations
Q = 32           # per-encoder feature width
TILE = 512       # nodes per PSUM tile
WIN = 128        # nodes per one-hot window
CH = 128         # edges per chunk (matmul contraction dim)
SB = 16          # chunks per one-hot build (16 * 128 = 2048 columns)
MAXG = 32        # max chunks per dma_gather instruction

F32 = mybir.dt.float32
BF16 = mybir.dt.bfloat16
I16 = mybir.dt.int16

# dtype knobs
XDT = mybir.dt.bfloat16    # x gather-table dtype (+ one-hot S, iota)
XDT_NP = ml_dtypes.bfloat16
EDT = mybir.dt.bfloat16    # encoder des/tweet input dtype
EDT_NP = ml_dtypes.bfloat16

NEG_SLOPE = 0.01


# ---------------------------------------------------------------------------
# host-side schedule builder
# ---------------------------------------------------------------------------

def _build_schedule(edge_index, edge_type, npc, npad):
    """Bucket edges into (tile, rel, bank, window) chunks.

    Returns dict with:
      C[nt, R, nbank, nw]   chunks per bucket (max over cores)
      groups                list of (nidx, idx_col_off) per dma_gather, in
                            chunk order; group g covers chunks
                            [gc0[g], gc0[g]+nidx//CH)
      src16[NC, 128, nchp]  bank-relative int16 gather row per chunk slot
      idxw[NC, 128, ncols]  wrapped+replicated idx array for dma_gather
      seg[NC, 128, nchp]    window-local one-hot id (-1 pad)
      recip[NC, nt*R*TILE]  1/deg per mean column
      bank_rows, nbank
    """
    nt = npad // TILE
    nw = TILE // WIN
    nfull = NCORES * npad
    bank_rows = (32767 // npad) * npad if npad <= 32767 else 0
    assert bank_rows > 0
    nbank = -(-nfull // bank_rows)

    src = np.asarray(edge_index[0], dtype=np.int64)
    dst = np.asarray(edge_index[1], dtype=np.int64)
    et = np.asarray(edge_type, dtype=np.int64)

    owner = dst // npc
    local = dst - owner * npc
    so = src // npc
    grow = so * npad + (src - so * npc)
    bank = grow // bank_rows
    gw = local // WIN                    # global window in [0, npad/WIN)
    t_of = local // TILE
    w_of = (local % TILE) // WIN

    # counts per (core, t, rel, bank, w)
    key = ((((owner * nt + t_of) * R + et) * nbank + bank) * nw + w_of)
    nkey = NCORES * nt * R * nbank * nw
    cnt = np.bincount(key, minlength=nkey).reshape(NCORES, nt, R, nbank, nw)
    C = np.maximum(np.ceil(cnt / CH).astype(np.int64).max(axis=0), 0)  # [nt,R,nbank,nw]
    # every (t, r, w) needs at least one chunk so PSUM gets written
    empty = C.sum(axis=2) == 0                     # [nt, R, nw]
    if empty.any():
        ti, ri, wi = np.nonzero(empty)
        C[ti, ri, 0, wi] = 1

    # segment counts (used for the per-edge 1/deg array below)
    skey = (owner * npc + local) * R + et
    cnt_seg = np.bincount(skey, minlength=NCORES * npc * R).reshape(NCORES, npc, R)

    # chunk bases in program order (t, r, b, w)
    flat = C.ravel()
    base = np.zeros_like(flat)
    base[1:] = np.cumsum(flat)[:-1]
    base_trbw = base.reshape(C.shape)
    nch = int(flat.sum())
    nchp = nch  # groups are bank runs; no power-of-two padding needed

    # scatter edges into chunk slots; ascending src within each bucket gives
    # the gather engine monotone HBM addresses (better row locality)
    order = np.lexsort((grow, key))
    gs = np.zeros(nkey + 1, np.int64)
    np.cumsum(np.bincount(key, minlength=nkey), out=gs[1:])
    rank = np.arange(len(order)) - gs[key[order]]
    oe = order
    slot = (base_trbw[t_of[oe], et[oe], bank[oe], w_of[oe]] + rank // CH)
    part = rank % CH
    src16 = np.zeros((NCORES, CH, nchp), np.int16)
    seg = np.full((NCORES, CH, nchp), -1.0, np.float32)
    src16[owner[oe], part, slot] = (grow[oe] - bank[oe] * bank_rows).astype(np.int16)
    seg[owner[oe], part, slot] = (local[oe] - gw[oe] * WIN).astype(np.float32)
    # per-edge 1/deg: scaling the one-hot row by it turns the segment-sum
    # matmul into the segment-mean directly (pad slots scale a zero row)
    rce = np.zeros((NCORES, CH, nchp), np.float32)
    deg = cnt_seg[owner[oe], local[oe], et[oe]].astype(np.float32)
    rce[owner[oe], part, slot] = 1.0 / np.maximum(deg, 1.0)

    # gather groups: contiguous chunk runs within each (t, r, b), <= MAXG
    groups = []          # (nidx, idx_col_off, chunk0, bank)
    col_off = 0
    for t in range(nt):
        for r in range(R):
            for b in range(nbank):
                run = int(C[t, r, b].sum())
                c0 = int(base_trbw[t, r, b, 0])
                while run > 0:
                    g = min(run, MAXG)
                    groups.append((g * CH, col_off, c0, b))
                    col_off += g * CH // 16
                    c0 += g
                    run -= g

    # wrapped idx array
    idxw = np.zeros((NCORES, CH, col_off), np.int16)
    for nidx, co, c0, _b in groups:
        k = nidx // CH
        lin = src16[:, :, c0:c0 + k].transpose(0, 2, 1).reshape(NCORES, nidx)
        wr = lin.reshape(NCORES, nidx // 16, 16).transpose(0, 2, 1)  # [NC,16,cols]
        idxw[:, :, co:co + nidx // 16] = np.tile(wr, (1, 8, 1))

    return dict(C=C, groups=groups, src16=src16, idxw=idxw, seg=seg,
                rce=rce, bank_rows=bank_rows, nbank=nbank, nch=nch,
                nchp=nchp, ncols=col_off)


# ---------------------------------------------------------------------------
# wait legalization (this walrus accepts max 1 wait/instruction, 2 on EVSEM)
# ---------------------------------------------------------------------------

def legalize_waits(nc):
    n_fixed = 0
    for f in nc.m.functions:
        for bb in f.blocks:
            new_insts = []
            for ins in bb.instructions:
                si = ins.sync_info
                cap = 2 if type(ins).__name__ == "InstEventSemaphore" else 1
                if si is not None and si.on_wait and len(si.on_wait) > cap:
                    extra = list(si.on_wait[cap:])
                    keep = list(si.on_wait[:cap])
                    for k, w in enumerate(extra):
                        d = mybir.InstDrain(
                            name=f"{ins.name}-waitsplit-{k}",
                            engine=ins.engine,
                            sync_info=mybir.SyncInfo(on_wait=[w], on_update=[]),
                        )
                        nc.register_instruction(d, overwrite=True)
                        new_insts.append(d)
                    si.on_wait = keep
                    n_fixed += 1
                new_insts.append(ins)
            bb.instructions = new_insts
    return n_fixed


# ---------------------------------------------------------------------------
# device program
# ---------------------------------------------------------------------------

def _build_program(sched, npad, nfd, nft, variant=frozenset()):
    C = sched["C"]
    groups = sched["groups"]
    bank_rows = sched["bank_rows"]
    nbank = sched["nbank"]
    nchp = sched["nchp"]
    ncols = sched["ncols"]
    nt = npad // TILE
    nw = TILE // WIN
    nfull = NCORES * npad
    maxg_idx = max(g[0] for g in groups)

    nc = bass.Bass()

    dt_in = {}

    def din(name, shape, dt):
        dt_in[name] = nc.dram_tensor(name, shape, dt, kind="ExternalInput")
        return dt_in[name]

    desT = din("desT", [128, nfd * npad], EDT)
    tweT = din("tweT", [128, nft * npad], EDT)
    proT = din("proT", [16, npad], EDT)
    Wdes = din("Wdes", [nfd * 128, Q], EDT)
    Wtwe = din("Wtwe", [nft * 128, Q], EDT)
    Wpro = din("Wpro", [16, Q], EDT)
    bdes = din("bdes", [Q, 1], F32)
    btwe = din("btwe", [Q, 1], F32)
    bpro = din("bpro", [Q, 1], F32)
    Win_ = din("Win_", [3 * Q, D], F32)
    bin_ = din("bin_", [D, 1], F32)
    Wrel0 = din("Wrel0", [D, D], F32)
    Wrel1 = din("Wrel1", [D, D], F32)
    Wroot = din("Wroot", [D, D], EDT)
    brg = din("brg", [D, 1], F32)
    Wo1 = din("Wo1", [D, D], EDT)
    bo1 = din("bo1", [D, 1], F32)
    Wo2 = din("Wo2", [D, 2], EDT)
    bo2 = din("bo2", [2, 1], F32)
    iden = din("iden", [128, 128], EDT)
    iota_c = din("iota_c", [CH, SB * WIN], XDT)
    idx_d = din("idx_d", [CH, ncols], I16)
    segl = din("segl", [CH, nchp], XDT)
    rce_d = din("rce_d", [CH, nchp], XDT)
    zout = nc.dram_tensor("zout", [2, npad], F32, kind="ExternalOutput")

    ag1_in = nc.dram_tensor("ag1_in", [npad, D], XDT)
    ag2_in = nc.dram_tensor("ag2_in", [npad, D], XDT)
    x1_full = nc.dram_tensor("x1_full", [nfull, D], XDT, addr_space="Shared")
    x2_full = nc.dram_tensor("x2_full", [nfull, D], XDT, addr_space="Shared")

    rg = [list(range(NCORES))]

    with tile.TileContext(nc) as tc:
        with tc.tile_pool(name="wpool", bufs=1) as wp:
            nc.gpsimd.load_library(library_config.mlp)
            nidx_regs = {}
            for nidx, _, _, _ in groups:
                if nidx not in nidx_regs:
                    nidx_regs[nidx] = nc.gpsimd.to_reg(nidx)

            ident = wp.tile([128, 128], EDT, tag="ident")
            nc.sync.dma_start(out=ident[:], in_=iden[:])
            iota_sb = wp.tile([CH, SB * WIN], XDT, tag="iota")
            nc.sync.dma_start(out=iota_sb[:], in_=iota_c[:])
            # idx/seg/recip tables are identical for both RGCN layers: load once.
            idx_sb = wp.tile([CH, ncols], I16, tag="idxbig")
            nc.sync.dma_start(out=idx_sb[:], in_=idx_d[:])
            seg_sb = wp.tile([CH, nchp], XDT, tag="segbig")
            nc.sync.dma_start(out=seg_sb[:], in_=segl[:])
            rce_sb = wp.tile([CH, nchp], XDT, tag="rcebig")
            nc.sync.dma_start(out=rce_sb[:], in_=rce_d[:])

            wdes_sb = wp.tile([128, nfd * Q], EDT, tag="wdes")
            nc.sync.dma_start(out=wdes_sb[:].rearrange("p (f q) -> p f q", q=Q),
                              in_=Wdes[:].rearrange("(f p) q -> p f q", p=128))
            wtwe_sb = wp.tile([128, nft * Q], EDT, tag="wtwe")
            nc.sync.dma_start(out=wtwe_sb[:].rearrange("p (f q) -> p f q", q=Q),
                              in_=Wtwe[:].rearrange("(f p) q -> p f q", p=128))
            wpro_sb = wp.tile([16, Q], EDT, tag="wpro")
            nc.sync.dma_start(out=wpro_sb[:], in_=Wpro[:])
            win_sb = wp.tile([3 * Q, D], F32, tag="win")
            nc.sync.dma_start(out=win_sb[:], in_=Win_[:])
            wrel0_sb = wp.tile([D, D], F32, tag="wrel0")
            nc.sync.dma_start(out=wrel0_sb[:], in_=Wrel0[:])
            wrel1_sb = wp.tile([D, D], F32, tag="wrel1")
            nc.sync.dma_start(out=wrel1_sb[:], in_=Wrel1[:])
            wroot_sb = wp.tile([D, D], EDT, tag="wroot")
            nc.sync.dma_start(out=wroot_sb[:], in_=Wroot[:])
            wo1_sb = wp.tile([D, D], EDT, tag="wo1")
            nc.sync.dma_start(out=wo1_sb[:], in_=Wo1[:])
            wo2_sb = wp.tile([D, 2], EDT, tag="wo2")
            nc.sync.dma_start(out=wo2_sb[:], in_=Wo2[:])
            bias_sb = {}
            for nm in ("bdes", "btwe", "bpro", "bin_", "brg", "bo1", "bo2"):
                t_ = dt_in[nm]
                bt_ = wp.tile(list(t_.shape), F32, tag=nm)
                nc.sync.dma_start(out=bt_[:], in_=t_[:])
                bias_sb[nm] = bt_

            # ---------- phase 1: encoder ----------
            # x1/x2 tiles [D, TILE] bf16 persist in SBUF across the layers
            # (root-term inputs) -- no DRAM roundtrip.
            x1_tiles = {}
            x2_tiles = {}
            with tc.tile_pool(name="enc_in", bufs=3) as ip, \
                 tc.tile_pool(name="enc_h", bufs=2) as hp, \
                 tc.tile_pool(name="enc_ps", bufs=2, space="PSUM") as pp, \
                 tc.tile_pool(name="enc_tr", bufs=2, space="PSUM") as tp, \
                 tc.tile_pool(name="enc_out", bufs=2) as op_:
                for t in range(nt):
                    sl = slice(t * TILE, (t + 1) * TILE)
                    hT = hp.tile([3 * Q, TILE], F32, tag="hT")
                    for pi, (td, wsb, bnm, nf) in enumerate((
                            (desT, wdes_sb, "bdes", nfd),
                            (tweT, wtwe_sb, "btwe", nft),
                            (proT, wpro_sb, "bpro", 1))):
                        ps = pp.tile([Q, TILE], F32, tag="enc_ps")
                        if nf == 1:
                            xt = ip.tile([16, TILE], EDT, tag="pro_in")
                            nc.sync.dma_start(out=xt[:], in_=td[:, sl])
                            nc.tensor.matmul(out=ps[:], lhsT=wsb[:], rhs=xt[:],
                                             start=True, stop=True)
                        else:
                            xt = ip.tile([128, nf * TILE], EDT, tag="enc_in")
                            nc.sync.dma_start(
                                out=xt[:].rearrange("p (f n) -> p f n", n=TILE),
                                in_=td[:].rearrange(
                                    "p (f n) -> p f n", n=npad)[:, :, sl])
                            for f in range(nf):
                                nc.tensor.matmul(
                                    out=ps[:], lhsT=wsb[:, f * Q:(f + 1) * Q],
                                    rhs=xt[:, f * TILE:(f + 1) * TILE],
                                    start=(f == 0), stop=(f == nf - 1))
                        nc.scalar.activation(
                            out=hT[pi * Q:(pi + 1) * Q, :], in_=ps[:],
                            func=mybir.ActivationFunctionType.Lrelu,
                            bias=bias_sb[bnm][:], alpha=NEG_SLOPE)
                    psx = pp.tile([D, TILE], F32, tag="enc_psx")
                    nc.tensor.matmul(out=psx[:], lhsT=win_sb[:], rhs=hT[:],
                                     start=True, stop=True)
                    x1t_sb = wp.tile([D, TILE], EDT, tag=f"x1t_{t}")
                    nc.scalar.activation(
                        out=x1t_sb[:], in_=psx[:],
                        func=mybir.ActivationFunctionType.Lrelu,
                        bias=bias_sb["bin_"][:], alpha=NEG_SLOPE)
                    x1_tiles[t] = x1t_sb
                    pst = tp.tile([128, TILE], EDT, tag="enc_tr")
                    for j in range(TILE // 128):
                        nc.tensor.transpose(out=pst[:, j * 128:(j + 1) * 128],
                                            in_=x1t_sb[:, j * 128:(j + 1) * 128],
                                            identity=ident[:])
                    row_sb = op_.tile([128, TILE], XDT, tag="row")
                    nc.vector.tensor_copy(out=row_sb[:], in_=pst[:])
                    nc.sync.dma_start(
                        out=ag1_in[sl, :].rearrange("(j p) d -> p j d", p=128),
                        in_=row_sb[:].rearrange("p (j d) -> p j d", d=D))

            # ---------- all-gather x1 ----------
            if "noag" not in variant:
                nc.gpsimd.collective_compute(
                    "AllGather", mybir.AluOpType.bypass, replica_groups=rg,
                    ins=[ag1_in[:]], outs=[x1_full[:]])

            # ---------- RGCN layers ----------
            def rgcn_layer(x_full, xtiles, sink):
                with tc.tile_pool(name="g_dat", bufs=4) as gdp, \
                     tc.tile_pool(name="g_s", bufs=3) as sp_, \
                     tc.tile_pool(name="l_mean", bufs=2, space="PSUM") as mp, \
                     tc.tile_pool(name="l_out", bufs=2, space="PSUM") as lop, \
                     tc.tile_pool(name="l_sb", bufs=4) as lsb:
                    gi = 0          # group index
                    gt = None       # current gather tile
                    g_start = 0     # chunk index where current group starts
                    g_end = 0
                    st_t = None     # current one-hot tile
                    s_start = 0
                    s_end = 0
                    c = 0

                    def ensure(c):
                        nonlocal gi, gt, g_start, g_end, st_t, s_start, s_end
                        if c >= g_end:
                            nidx, co, c0, b = groups[gi]
                            assert c0 == c, (c0, c)
                            gt_ = gdp.tile([CH, maxg_idx * D // CH], XDT, tag="gat")
                            kd = nidx // CH * D
                            if "nodma" in variant:
                                pass
                            elif "seqdma" in variant:
                                nc.sync.dma_start(
                                    out=gt_[:, :kd].rearrange(
                                        "p (k d) -> p k d", d=D),
                                    in_=x_full[:nidx, :].rearrange(
                                        "(p k) d -> p k d", p=CH))
                            else:
                                k16 = nidx // 16
                                nc.gpsimd.dma_gather(
                                    gt_[:, :kd].rearrange("p (k d) -> p k d", d=D),
                                    x_full[b * bank_rows:
                                           min((b + 1) * bank_rows, nfull), :],
                                    idx_sb[:, co:co + k16], nidx,
                                    nidx_regs[nidx], D,
                                    single_packet=False)
                            if "nodma" not in variant:
                                # scale each gathered row by 1/deg(dst): the
                                # accumulated segment matmul then yields the
                                # mean without touching the one-hot
                                nc.vector.tensor_tensor(
                                    out=gt_[:, :kd].rearrange(
                                        "p (k d) -> p k d", d=D),
                                    in0=gt_[:, :kd].rearrange(
                                        "p (k d) -> p k d", d=D),
                                    in1=rce_sb[:, c0:c0 + nidx // CH, None]
                                    .to_broadcast([CH, nidx // CH, D]),
                                    op=mybir.AluOpType.mult)
                            gt = gt_
                            g_start = c
                            g_end = c + nidx // CH
                            gi += 1
                        if c >= s_end:
                            nsb = min(SB, nchp - c)
                            s_t = sp_.tile([CH, SB * WIN], XDT, tag="s")
                            if "nooh" not in variant:
                                nc.vector.tensor_tensor(
                                    out=s_t[:, :nsb * WIN].rearrange(
                                        "p (c w) -> p c w", w=WIN),
                                    in0=iota_sb[:, :nsb * WIN].rearrange(
                                        "p (c w) -> p c w", w=WIN),
                                    in1=seg_sb[:, c:c + nsb, None].to_broadcast(
                                        [CH, nsb, WIN]),
                                    op=mybir.AluOpType.is_equal)
                            st_t = s_t
                            s_start = c
                            s_end = c + nsb

                    skip_mm = ("nomm" in variant or "nooh" in variant
                               or "nodma" in variant or "noloop" in variant)
                    for t in range(nt):
                        sl = slice(t * TILE, (t + 1) * TILE)
                        mean_sb = []
                        for r in range(R):
                            psm = mp.tile([D, TILE], F32, tag="mean_ps")
                            if skip_mm:
                                nc.tensor.matmul(
                                    out=psm[:1, :1], lhsT=wroot_sb[:, :1],
                                    rhs=wroot_sb[:, :1], start=True, stop=True)
                            # (start, stop) flags per window across banks
                            nchunks_w = C[t, r].sum(axis=0)      # [nw]
                            done_w = np.zeros(nw, np.int64)
                            for b in range(nbank):
                                for w in range(nw):
                                    for k in range(int(C[t, r, b, w])):
                                        if "noloop" in variant:
                                            c += 1
                                            continue
                                        ensure(c)
                                        go = c - g_start
                                        so = c - s_start
                                        if not skip_mm:
                                            nc.tensor.matmul(
                                                out=psm[:, w * WIN:(w + 1) * WIN],
                                                lhsT=gt[:, go * D:(go + 1) * D],
                                                rhs=st_t[:, so * WIN:(so + 1) * WIN],
                                                start=(done_w[w] == 0),
                                                stop=(done_w[w] == nchunks_w[w] - 1))
                                        done_w[w] += 1
                                        c += 1
                            msb = lsb.tile([D, TILE], F32, tag="mean_sb")
                            nc.vector.tensor_copy(out=msb[:], in_=psm[:])
                            mean_sb.append(msb)
                        pso = lop.tile([D, TILE], F32, tag="out_ps")
                        nc.tensor.matmul(out=pso[:], lhsT=wroot_sb[:],
                                         rhs=xtiles[t][:], start=True, stop=False)
                        nc.tensor.matmul(out=pso[:], lhsT=wrel0_sb[:],
                                         rhs=mean_sb[0][:], start=False, stop=False)
                        nc.tensor.matmul(out=pso[:], lhsT=wrel1_sb[:],
                                         rhs=mean_sb[1][:], start=False, stop=True)
                        sink(t, pso)

            # layer 1 sink
            def make_l1_sink(osbp, trp):
                def l1_sink(t, pso):
                    sl = slice(t * TILE, (t + 1) * TILE)
                    osb = wp.tile([D, TILE], EDT, tag=f"x2t_{t}")
                    nc.vector.tensor_scalar(
                        out=osb[:], in0=pso[:], scalar1=bias_sb["brg"][:],
                        scalar2=None, op0=mybir.AluOpType.add)
                    x2_tiles[t] = osb
                    pst = trp.tile([128, TILE], EDT, tag="l1_tr")
                    for j in range(TILE // 128):
                        nc.tensor.transpose(out=pst[:, j * 128:(j + 1) * 128],
                                            in_=osb[:, j * 128:(j + 1) * 128],
                                            identity=ident[:])
                    row_sb = osbp.tile([128, TILE], XDT, tag="l1_row")
                    nc.vector.tensor_copy(out=row_sb[:], in_=pst[:])
                    nc.sync.dma_start(
                        out=ag2_in[sl, :].rearrange("(j p) d -> p j d", p=128),
                        in_=row_sb[:].rearrange("p (j d) -> p j d", d=D))
                return l1_sink

            with tc.tile_pool(name="l1_osb", bufs=3) as osbp, \
                 tc.tile_pool(name="l1_tr", bufs=2, space="PSUM") as trp:
                rgcn_layer(x1_full, x1_tiles, make_l1_sink(osbp, trp))

            # ---------- all-gather x2 ----------
            if "noag" not in variant:
                nc.gpsimd.collective_compute(
                    "AllGather", mybir.AluOpType.bypass, replica_groups=rg,
                    ins=[ag2_in[:]], outs=[x2_full[:]])

            # layer 2 + head
            def make_l2_sink(sbp, psp, ps2p):
                def l2_sink(t, pso):
                    sl = slice(t * TILE, (t + 1) * TILE)
                    o2 = sbp.tile([D, TILE], EDT, tag="o2")
                    nc.vector.tensor_scalar(
                        out=o2[:], in0=pso[:], scalar1=bias_sb["brg"][:],
                        scalar2=None, op0=mybir.AluOpType.add)
                    psz = psp.tile([D, TILE], F32, tag="z1ps")
                    nc.tensor.matmul(out=psz[:], lhsT=wo1_sb[:], rhs=o2[:],
                                     start=True, stop=True)
                    z1 = sbp.tile([D, TILE], EDT, tag="z1")
                    nc.scalar.activation(out=z1[:], in_=psz[:],
                                         func=mybir.ActivationFunctionType.Lrelu,
                                         bias=bias_sb["bo1"][:], alpha=NEG_SLOPE)
                    psz2 = ps2p.tile([2, TILE], F32, tag="z2ps")
                    nc.tensor.matmul(out=psz2[:], lhsT=wo2_sb[:], rhs=z1[:],
                                     start=True, stop=True)
                    z2 = sbp.tile([2, TILE], F32, tag="z2")
                    nc.vector.tensor_scalar(
                        out=z2[:], in0=psz2[:], scalar1=bias_sb["bo2"][:],
                        scalar2=None, op0=mybir.AluOpType.add)
                    nc.sync.dma_start(out=zout[:, sl], in_=z2[:])
                return l2_sink

            with tc.tile_pool(name="l2_sb", bufs=4) as sbp, \
                 tc.tile_pool(name="l2_ps", bufs=2, space="PSUM") as psp, \
                 tc.tile_pool(name="l2_ps2", bufs=2, space="PSUM") as ps2p:
                rgcn_layer(x2_full, x2_tiles, make_l2_sink(sbp, psp, ps2p))

    legalize_waits(nc)
    library_overlay.lower_extended_insts(nc)
    return nc


# ---------------------------------------------------------------------------
# SPMD runner (mirrors bass2jax.run_bass_via_pjrt, with optional re-run timing)
# ---------------------------------------------------------------------------

BENCH_REPEATS = 0          # async executions per timed batch (set by test harness)
BENCH_BATCHES = 2          # timed batches; LAST_BENCH_SECONDS = min per-iter
LAST_BENCH_SECONDS = None  # best amortized per-execution wall-clock
LAST_BENCH_ALL = None      # per-batch amortized times (noise diagnosis)
_VARIANT = frozenset()     # structural ablations for perf bisection (bench.py)


def _run_spmd(nc, in_maps, n_cores):
    import time
    import jax
    from jax.sharding import Mesh, PartitionSpec
    from jax.experimental.shard_map import shard_map
    from concourse import bass2jax
    from concourse.bass2jax import _bass_exec_p, install_neuronx_cc_hook

    global LAST_BENCH_SECONDS
    install_neuronx_cc_hook()

    partition_name = nc.partition_id_tensor.name if nc.partition_id_tensor else None
    in_names, out_names, out_avals, zero_outs = [], [], [], []
    for alloc in nc.m.functions[0].allocations:
        if not isinstance(alloc, mybir.MemoryLocationSet):
            continue
        name = alloc.memorylocations[0].name
        if alloc.kind == "ExternalInput":
            if name != partition_name:
                in_names.append(name)
        elif alloc.kind == "ExternalOutput":
            shape = tuple(alloc.tensor_shape)
            dtype = mybir.dt.np(alloc.dtype)
            out_names.append(name)
            out_avals.append(jax.core.ShapedArray(shape, dtype))
            zero_outs.append(np.zeros(shape, dtype))
    n_params = len(in_names)
    n_outs = len(out_avals)
    all_in_names = list(in_names) + list(out_names)
    if partition_name is not None:
        all_in_names.append(partition_name)

    def _body(*args):
        operands = list(args)
        if partition_name is not None:
            operands.append(bass2jax.partition_id_tensor())
        outs = _bass_exec_p.bind(
            *operands,
            out_avals=tuple(out_avals),
            in_names=tuple(all_in_names),
            out_names=tuple(out_names),
            lowering_input_output_aliases=(),
            sim_require_finite=True,
            sim_require_nnan=True,
            nc=nc,
        )
        return tuple(outs)

    devices = jax.devices()[:n_cores]
    mesh = Mesh(np.asarray(devices), ("core",))
    in_specs = (PartitionSpec("core"),) * (n_params + n_outs)
    out_specs = (PartitionSpec("core"),) * n_outs
    sharded = jax.jit(
        shard_map(_body, mesh=mesh, in_specs=in_specs, out_specs=out_specs,
                  check_rep=False),
        keep_unused=True)

    from jax.sharding import NamedSharding
    shard = NamedSharding(mesh, PartitionSpec("core"))
    concat_in = [
        np.concatenate([np.asarray(in_maps[c][nm]) for c in range(n_cores)], axis=0)
        for nm in in_names
    ]
    concat_in = [jax.device_put(a, shard) for a in concat_in]
    zeros_dev = [
        jax.device_put(np.zeros((n_cores * z.shape[0], *z.shape[1:]), z.dtype), shard)
        for z in zero_outs
    ]

    out_arrs = sharded(*concat_in, *zeros_dev)
    out_arrs = [np.asarray(a) for a in out_arrs]

    if BENCH_REPEATS > 0:
        # Amortized-throughput timing: the axon tunnel adds 60-80 ms of
        # round-trip latency to a synchronous dispatch, dwarfing device
        # time.  Dispatches pipeline, so submit a batch asynchronously and
        # block once; per-iteration time then approaches true device time.
        times = []
        for _ in range(BENCH_BATCHES):
            t0 = time.perf_counter()
            outs = [sharded(*concat_in, *zeros_dev)
                    for _ in range(BENCH_REPEATS)]
            jax.block_until_ready(outs)
            times.append((time.perf_counter() - t0) / BENCH_REPEATS)
        LAST_BENCH_SECONDS = min(times)
        global LAST_BENCH_ALL
        LAST_BENCH_ALL = times

    return [
        {nm: out_arrs[i].reshape(n_cores, *out_avals[i].shape)[c]
         for i, nm in enumerate(out_names)}
        for c in range(n_cores)
    ]


# ---------------------------------------------------------------------------
# entry point
# ---------------------------------------------------------------------------

def kernel(des, tweet, prop, edge_index, edge_type,
           W_des, b_des, W_tweet, b_tweet, W_prop, b_prop,
           W_in, b_in, W_rel, W_root, b_rgcn,
           W_o1, b_o1, W_o2, b_o2):
    des = np.asarray(des)
    tweet = np.asarray(tweet)
    prop = np.asarray(prop)
    n = des.shape[0]
    assert n % NCORES == 0
    npc = n // NCORES
    nt = (npc + TILE - 1) // TILE
    npad = nt * TILE
    nfd = des.shape[1] // 128
    nft = tweet.shape[1] // 128

    sched = _build_schedule(edge_index, edge_type, npc, npad)
    nc = _build_program(sched, npad, nfd, nft, _VARIANT)

    iota_v = np.tile(np.arange(WIN, dtype=np.float32), (CH, SB)).astype(XDT_NP)
    wpro_pad = np.zeros((16, Q), np.float32)
    wpro_pad[:prop.shape[1]] = np.asarray(W_prop, np.float32)
    weights_common = dict(
        Wdes=np.asarray(W_des).astype(EDT_NP), Wtwe=np.asarray(W_tweet).astype(EDT_NP),
        Wpro=wpro_pad.astype(EDT_NP),
        bdes=np.asarray(b_des, np.float32).reshape(-1, 1),
        btwe=np.asarray(b_tweet, np.float32).reshape(-1, 1),
        bpro=np.asarray(b_prop, np.float32).reshape(-1, 1),
        Win_=np.asarray(W_in, np.float32),
        bin_=np.asarray(b_in, np.float32).reshape(-1, 1),
        Wrel0=np.asarray(W_rel[0], np.float32), Wrel1=np.asarray(W_rel[1], np.float32),
        Wroot=np.asarray(W_root).astype(EDT_NP),
        brg=np.asarray(b_rgcn, np.float32).reshape(-1, 1),
        Wo1=np.asarray(W_o1).astype(EDT_NP),
        bo1=np.asarray(b_o1, np.float32).reshape(-1, 1),
        Wo2=np.asarray(W_o2).astype(EDT_NP),
        bo2=np.asarray(b_o2, np.float32).reshape(-1, 1),
        iden=np.eye(128, dtype=np.float32).astype(EDT_NP),
        iota_c=iota_v,
    )

    in_maps = []
    for cc in range(NCORES):
        rs, re = cc * npc, (cc + 1) * npc
        dpad = np.zeros((npad, des.shape[1]), np.float32)
        dpad[:npc] = des[rs:re]
        tpad = np.zeros((npad, tweet.shape[1]), np.float32)
        tpad[:npc] = tweet[rs:re]
        ppad = np.zeros((npad, 16), np.float32)
        ppad[:npc, :prop.shape[1]] = prop[rs:re]
        m = dict(weights_common)
        nfd_ = des.shape[1] // 128
        nft_ = tweet.shape[1] // 128
        # [128, f*npad + col] layout: one strided DMA loads a whole tile
        m["desT"] = np.ascontiguousarray(
            dpad.T.reshape(nfd_, 128, npad).transpose(1, 0, 2)
            .reshape(128, nfd_ * npad)).astype(EDT_NP)
        m["tweT"] = np.ascontiguousarray(
            tpad.T.reshape(nft_, 128, npad).transpose(1, 0, 2)
            .reshape(128, nft_ * npad)).astype(EDT_NP)
        m["proT"] = np.ascontiguousarray(ppad.T).astype(EDT_NP)
        m["idx_d"] = np.ascontiguousarray(sched["idxw"][cc])
        m["segl"] = np.ascontiguousarray(sched["seg"][cc]).astype(XDT_NP)
        m["rce_d"] = np.ascontiguousarray(sched["rce"][cc]).astype(XDT_NP)
        in_maps.append(m)

    results = _run_spmd(nc, in_maps, NCORES)

    z = np.concatenate(
        [results[cc]["zout"][:, :npc].T for cc in range(NCORES)], axis=0)
    z = z - z.max(axis=1, keepdims=True)
    ez = np.exp(z)
    out = ez / ez.sum(axis=1, keepdims=True)
    return out.astype(np.float32)

